# revision 28
# baseline (speedup 1.0000x reference)
"""Causal self-attention (B=4, T=2048, D=1024, H=16) on 8 Trainium2 NeuronCores.

Sharding: batch x head-group hybrid. Core c handles batch b = c % 4 and head
group g = c // 4 (heads 8g..8g+7). Each core computes its heads' attention and
a partial output projection [T, D]; the host sums the two head-group partials
per batch (the all-reduce of the output projection, done at gather time).

Per-core kernel, all matmuls in float32r (~1.5e-4 rms rel err, 4x fp32 rate):

  The instruction stream is emitted explicitly interleaved: QKV-projection
  matmul groups for token chunk t+1 and the (deferred) output projection of
  chunk t-1 are spread between the attention iterations of chunk t, so the
  PE never stalls on ScalarE's exp and the HAM clock-gate stays at 2.4 GHz.

  - qT/kT are produced channel-major [ch, T] with head pairs packed in
    64-partition halves; the two K=64 score matmuls of a pair run
    concurrently in PE row groups 0-1 / 2-3, writing one [128, 2, 512]
    PSUM pair tile that a single ScalarE exp consumes.
  - V is produced token-major with an appended ones*mask column, so the AV
    matmul emits the softmax denominator as row 64 of its PSUM output.
  - Causal staircase masks (slices of one [128, 896] tile) are multiplied on
    the diagonal 128x512 blocks only, alternating DVE / GpSimd.
  - Normalization: denominator row -> gpsimd partition-broadcast -> fast
    Newton reciprocal (64 partitions) -> multiply.
"""

import sys
import types

import numpy as np


def _ensure_axon_hooks_stub():
    # bass_utils imports antenv.axon_hooks when tracing is requested (e.g. via
    # a BASS_TRACE env); the module is absent in this image. Provide a stub
    # that reports "no hook" unless a harness already installed a real one.
    if "antenv.axon_hooks" in sys.modules:
        return
    mod = types.ModuleType("antenv.axon_hooks")
    _hook = [None]
    mod.set_axon_ntff_profile_hook = lambda h: _hook.__setitem__(0, h)
    mod.get_axon_ntff_profile_hook = lambda: _hook[0]
    sys.modules["antenv.axon_hooks"] = mod
    try:
        import antenv

        antenv.axon_hooks = mod
    except ImportError:
        pass


_ensure_axon_hooks_stub()

import concourse.mybir as mybir  # noqa: E402
import concourse.tile as tile  # noqa: E402
from concourse import bacc  # noqa: E402
from concourse.bass import ts  # noqa: E402
from concourse.bass_utils import run_bass_kernel_spmd  # noqa: E402

P = 128
B, T, D = 4, 2048, 1024
H, HD = 16, 64
HG = 8          # heads per group (per core)
DG = HG * HD    # 512 channels per group
KO = D // P    # 8 contraction chunks for the projections
TQ = 512        # token chunk (attention q tile and QKV free dim)
NQT = T // TQ   # 4
F32 = mybir.dt.float32
F32R = mybir.dt.float32r

_PROGRAM = None


def _merge(attn_items, filler_items):
    """Round-robin: spread filler emission evenly between attention items."""
    out = []
    na, nf = len(attn_items), len(filler_items)
    fi = 0
    for i, a in enumerate(attn_items):
        out.append(a)
        while fi < nf and (i + 1) * nf >= (fi + 1) * na:
            out.append(filler_items[fi])
            fi += 1
    out.extend(filler_items[fi:])
    return out


def _build_program():
    nc = bacc.Bacc(None, target_bir_lowering=False, debug=False)

    xT = nc.dram_tensor("xT", [D, T], F32R, kind="ExternalInput")
    wqT = nc.dram_tensor("wqT", [D, DG], F32R, kind="ExternalInput")
    wkT = nc.dram_tensor("wkT", [D, DG], F32R, kind="ExternalInput")
    wvT = nc.dram_tensor("wvT", [D, DG], F32R, kind="ExternalInput")
    wpT = nc.dram_tensor("wpT", [DG, D], F32R, kind="ExternalInput")
    dmask = nc.dram_tensor("dmask", [P, 512], F32R, kind="ExternalInput")
    amask = nc.dram_tensor("amask", [P, T // P], F32, kind="ExternalInput")
    out = nc.dram_tensor("out", [T, D], F32, kind="ExternalOutput")

    xT3 = xT.ap().rearrange("(ko p) t -> p ko t", p=P)
    wq3 = wqT.ap().rearrange("(ko p) c -> p ko c", p=P)
    wk3 = wkT.ap().rearrange("(ko p) c -> p ko c", p=P)
    wv3 = wvT.ap().rearrange("(ko p) c -> p ko c", p=P)
    wp3 = wpT.ap().rearrange("(co p) d -> p co d", p=P)

    with tile.TileContext(nc) as tc:
        with tc.tile_pool(name="const", bufs=1) as cpool, \
             tc.tile_pool(name="w", bufs=1) as wpool, \
             tc.tile_pool(name="kgp", bufs=4) as kgp, \
             tc.tile_pool(name="vap", bufs=4) as vap, \
             tc.tile_pool(name="qgp", bufs=2) as qpool, \
             tc.tile_pool(name="xp", bufs=1) as xpool, \
             tc.tile_pool(name="attn", bufs=2) as apool, \
             tc.tile_pool(name="expp", bufs=4) as epool, \
             tc.tile_pool(name="divp", bufs=1) as dpool, \
             tc.tile_pool(name="outp", bufs=1) as opool, \
             tc.tile_pool(name="flow", bufs=2, space="PSUM") as flow, \
             tc.tile_pool(name="scp", bufs=2, space="PSUM") as scp, \
             tc.tile_pool(name="avp", bufs=2, space="PSUM") as avp:

            dmask_sb = cpool.tile([P, 512], F32R, tag="dmask")
            nc.sync.dma_start(dmask_sb[:], dmask.ap())
            amask_sb = cpool.tile([P, T // P], F32, tag="amask")
            nc.sync.dma_start(amask_sb[:], amask.ap())

            wq_sb = wpool.tile([P, KO, DG], F32R, tag="wq")
            wk_sb = wpool.tile([P, KO, DG], F32R, tag="wk")
            wv_sb = wpool.tile([P, KO, DG], F32R, tag="wv")
            wp_sb = wpool.tile([P, DG // P, D], F32R, tag="wp")
            # Both HWDGE queues stream concurrently; wq + x(0) land first so
            # the first q-projection matmuls start as early as possible.
            for kk in range(KO):
                eng = nc.sync if kk % 2 == 0 else nc.scalar
                eng.dma_start(wq_sb[:, kk], wq3[:, kk])
            x0_sb = xpool.tile([P, KO, TQ], F32R, tag="x", name="x0")
            for kk in range(KO):
                eng = nc.sync if kk % 2 == 1 else nc.scalar
                eng.dma_start(x0_sb[:, kk], xT3[:, kk, ts(0, TQ)])
            for kk in range(KO):
                eng = nc.sync if kk % 2 == 0 else nc.scalar
                eng.dma_start(wk_sb[:, kk], wk3[:, kk])
            for kk in range(KO):
                eng = nc.sync if kk % 2 == 1 else nc.scalar
                eng.dma_start(wv_sb[:, kk], wv3[:, kk])
            for co in range(DG // P):
                eng = nc.sync if co % 2 == 0 else nc.scalar
                eng.dma_start(wp_sb[:, co], wp3[:, co])

            kg = [None] * NQT     # per-chunk kT tiles [P, hp, TQ]
            va = [None] * NQT     # per-chunk v_aug tiles [P, h, kt2, 65]
            qg = [None] * NQT
            attn_qt = [None] * NQT
            mask_rr = [0]         # round-robin DVE/GpSimd for mask multiplies

            def qkv_items(tc4, x_pre=None):
                """QKV projection for 512-token chunk tc4, as emission items."""
                items = []
                if x_pre is not None:
                    x_sb = x_pre
                else:
                    x_sb = xpool.tile([P, KO, TQ], F32R, tag="x", name=f"x{tc4}")
                qg[tc4] = qpool.tile([P, NQT, TQ], F32R, tag="qg", name=f"qg{tc4}")
                kg[tc4] = kgp.tile([P, NQT, TQ], F32R, tag="kg", name=f"kg{tc4}")
                va[tc4] = vap.tile([P, HG, NQT, HD + 1], F32R, tag="va", name=f"va{tc4}")

                def load_x():
                    for kk in range(KO):
                        nc.sync.dma_start(x_sb[:, kk], xT3[:, kk, ts(tc4, TQ)])
                if x_pre is None:
                    items.append(load_x)

                def qk_group(w_sb, dst, cc):
                    def go():
                        ps = flow.tile([P, TQ], F32, tag="flow")
                        for kk in range(KO):
                            nc.tensor.matmul(
                                ps[:], w_sb[:, kk, ts(cc, P)], x_sb[:, kk],
                                start=(kk == 0), stop=(kk == KO - 1),
                            )
                        nc.vector.tensor_copy(dst[:, cc, :], ps[:])
                    return go

                def v_group(tt2):
                    def go():
                        ps = flow.tile([P, HG, HD], F32, tag="flow")
                        for kk in range(KO):
                            nc.tensor.matmul(
                                ps.rearrange("p h d -> p (h d)"),
                                x_sb[:, kk, ts(tt2, P)],
                                wv_sb[:, kk],
                                start=(kk == 0), stop=(kk == KO - 1),
                            )
                        am = amask_sb[:, 4 * tc4 + tt2 : 4 * tc4 + tt2 + 1]
                        nc.vector.tensor_scalar_mul(
                            va[tc4][:, :, tt2, 0:HD], ps[:], am,
                        )
                        nc.vector.tensor_copy(
                            va[tc4][:, :, tt2, HD : HD + 1],
                            am[:, None, :].to_broadcast([P, HG, 1]),
                        )
                    return go

                for cc in range(NQT):
                    items.append(qk_group(wq_sb, qg[tc4], cc))
                for cc in range(NQT):
                    items.append(qk_group(wk_sb, kg[tc4], cc))
                for tt2 in range(NQT):
                    items.append(v_group(tt2))
                return items

            def attn_hp_items(qt, hp):
                """Attention for (q chunk qt, head pair hp), software-pipelined:
                scores+exp for kt are emitted one step ahead of the AV matmuls
                for kt-1, so the PE never sits directly behind exp."""
                items = []
                if attn_qt[qt] is None:
                    attn_qt[qt] = apool.tile(
                        [P, NQT, TQ], F32R, tag="attn", name=f"attn{qt}")
                nkt = 4 * (qt + 1)
                av = [
                    avp.tile([P, TQ], F32, tag="av", name=f"av{qt}_{hp}_{par}")
                    for par in range(2)
                ]
                ex = [None] * nkt

                def scores(kt, ex=ex):
                    def go():
                        sc = scp.tile([P, 2, TQ], F32, tag="sc")
                        for par in range(2):
                            rows = slice(64 * par, 64 * par + 64)
                            nc.tensor.matmul(
                                sc[:, par],
                                kg[kt // 4][rows, hp, ts(kt % 4, P)],
                                qg[qt][rows, hp, :],
                                start=True, stop=True,
                            )
                        e = epool.tile([P, 2, TQ], F32R, tag="exp")
                        o = kt - 4 * qt
                        if o < 0:
                            nc.scalar.activation(
                                e[:], sc[:],
                                mybir.ActivationFunctionType.Exp, scale=0.125,
                            )
                        else:
                            # diagonal: exp only live columns, staircase-mask
                            # the 128-wide triangle, zero the dead columns
                            c0 = 128 * o
                            nc.scalar.activation(
                                e[:, :, c0:TQ], sc[:, :, c0:TQ],
                                mybir.ActivationFunctionType.Exp, scale=0.125,
                            )
                            tri = dmask_sb[:, 384:512]
                            nc.vector.tensor_tensor(
                                e[:, :, c0 : c0 + 128], e[:, :, c0 : c0 + 128],
                                tri[:, None, :].to_broadcast([P, 2, 128]),
                                mybir.AluOpType.mult,
                            )
                            if o > 0:
                                nc.vector.memset(e[:, :, 0:c0].bitcast(mybir.dt.uint32), 0)
                        ex[kt] = e
                    return go

                def avmm(kt, av=av, ex=ex):
                    def go():
                        for par in range(2):
                            nc.tensor.matmul(
                                av[par][: HD + 1, :],
                                va[kt // 4][:, 2 * hp + par, kt % 4, :],
                                ex[kt][:, par],
                                start=(kt == 0), stop=(kt == nkt - 1),
                            )
                    return go

                def chain(fns):
                    def go():
                        for f in fns:
                            f()
                    return go

                items.append(scores(0))
                for kt in range(1, nkt):
                    items.append(chain([scores(kt), avmm(kt - 1)]))
                items.append(avmm(nkt - 1))

                def division():
                    def go():
                        for par in range(2):
                            den = dpool.tile([1, TQ], F32, tag="den")
                            nc.vector.tensor_copy(den[:], av[par][HD : HD + 1, :])
                            rb = dpool.tile([HD, TQ], F32, tag="rb")
                            nc.gpsimd.partition_broadcast(rb[:], den[:], channels=HD)
                            rec = dpool.tile([HD, TQ], F32, tag="rec")
                            nc.vector.reciprocal_approx_fast(rec[:], rb[:])
                            nc.vector.tensor_tensor(
                                attn_qt[qt][slice(64 * par, 64 * par + 64), hp, :],
                                av[par][0:HD, :], rec[:],
                                mybir.AluOpType.mult,
                            )
                    return go

                items.append(division())
                return items

            def outproj_items(qt):
                """Output projection for q chunk qt."""
                items = []

                def tt_group(tt2):
                    def go():
                        o_sb = opool.tile([P, D], F32, tag="osb")
                        for nb in range(D // TQ):
                            ps = flow.tile([P, TQ], F32, tag="flow")
                            for cc in range(DG // P):
                                nc.tensor.matmul(
                                    ps[:],
                                    attn_qt[qt][:, cc, ts(tt2, P)],
                                    wp_sb[:, cc, ts(nb, TQ)],
                                    start=(cc == 0), stop=(cc == DG // P - 1),
                                )
                            nc.vector.tensor_copy(o_sb[:, ts(nb, TQ)], ps[:])
                        nc.sync.dma_start(
                            out.ap()[ts(qt * NQT + tt2, P), :], o_sb[:]
                        )
                    return go

                for tt2 in range(NQT):
                    items.append(tt_group(tt2))
                return items

            # Emission schedule (engine queues execute in emission order, so
            # PE-filler work is placed where attention would stall on exp):
            #   qkv(0) | attn(0) x qkv(1) | attn(1) x [qkv(2), op(0)]
            #   | attn(2) x qkv(3) | attn(3,hp01) x op(1)
            #   | attn(3,hp23) x op(2) | op(3)
            def attn_qt_items(qt, hps):
                items = []
                for hp in hps:
                    items += attn_hp_items(qt, hp)
                return items

            for it in qkv_items(0, x_pre=x0_sb):
                it()
            for it in _merge(attn_qt_items(0, range(4)), qkv_items(1)):
                it()
            for it in _merge(attn_qt_items(1, range(4)),
                             qkv_items(2) + outproj_items(0)):
                it()
            for it in _merge(attn_qt_items(2, range(4)), qkv_items(3)):
                it()
            for it in _merge(attn_qt_items(3, [0, 1]), outproj_items(1)):
                it()
            for it in _merge(attn_qt_items(3, [2, 3]), outproj_items(2)):
                it()
            for it in outproj_items(NQT - 1):
                it()

    nc.compile()
    return nc


def _get_program():
    global _PROGRAM
    if _PROGRAM is None:
        _PROGRAM = _build_program()
    return _PROGRAM


def _staircase_mask() -> np.ndarray:
    # dmask[i, j] = 1.0 iff j >= i + 384; the slice [:, 384-128*o : 512]
    # give the four diagonal-block staircase masks.
    i = np.arange(P)[:, None]
    j = np.arange(512)[None, :]
    return (j >= i + 384).astype(np.float32)


def make_in_maps(x, attention_mask, w_qkv, w_proj):
    x = np.asarray(x, dtype=np.float32)
    attention_mask = np.asarray(attention_mask)
    w_qkv = np.asarray(w_qkv, dtype=np.float32)
    w_proj = np.asarray(w_proj, dtype=np.float32)
    dm = _staircase_mask()
    in_maps = []
    for c in range(8):
        g, b = c // 4, c % 4
        rows = slice(DG * g, DG * g + DG)
        in_maps.append({
            "xT": np.ascontiguousarray(x[b].T),
            "wqT": np.ascontiguousarray(w_qkv[0 * D :][rows].T),
            "wkT": np.ascontiguousarray(w_qkv[1 * D :][rows].T),
            "wvT": np.ascontiguousarray(w_qkv[2 * D :][rows].T),
            "wpT": np.ascontiguousarray(w_proj[:, rows].T),
            "dmask": dm,
            "amask": np.ascontiguousarray(
                attention_mask[b].astype(np.float32).reshape(T // P, P).T
            ),
        })
    return in_maps


def run_spmd(in_maps, **kwargs):
    nc = _get_program()
    return run_bass_kernel_spmd(nc, in_maps, list(range(8)), **kwargs)


def kernel(x, attention_mask, w_qkv, w_proj, n_heads):
    assert int(n_heads) == H
    in_maps = make_in_maps(x, attention_mask, w_qkv, w_proj)
    res = run_spmd(in_maps)
    parts = [res.results[c]["out"] for c in range(8)]
    return np.stack([parts[b] + parts[b + 4] for b in range(B)]).astype(np.float32)


# revision 29
# speedup vs baseline: 1.0149x; 1.0149x over previous
"""Causal self-attention (B=4, T=2048, D=1024, H=16) on 8 Trainium2 NeuronCores.

Sharding: batch x head-group hybrid. Core c handles batch b = c % 4 and head
group g = c // 4 (heads 8g..8g+7). Each core computes its heads' attention and
a partial output projection [T, D]; the host sums the two head-group partials
per batch (the all-reduce of the output projection, done at gather time).

Per-core kernel, all matmuls in float32r (~1.5e-4 rms rel err, 4x fp32 rate):

  The instruction stream is emitted explicitly interleaved: QKV-projection
  matmul groups for token chunk t+1 and the (deferred) output projection of
  chunk t-1 are spread between the attention iterations of chunk t, so the
  PE never stalls on ScalarE's exp and the HAM clock-gate stays at 2.4 GHz.

  - qT/kT are produced channel-major [ch, T] with head pairs packed in
    64-partition halves; the two K=64 score matmuls of a pair run
    concurrently in PE row groups 0-1 / 2-3, writing one [128, 2, 512]
    PSUM pair tile that a single ScalarE exp consumes.
  - V is produced token-major with an appended ones*mask column, so the AV
    matmul emits the softmax denominator as row 64 of its PSUM output.
  - Causal staircase masks (slices of one [128, 896] tile) are multiplied on
    the diagonal 128x512 blocks only, alternating DVE / GpSimd.
  - Normalization: denominator row -> gpsimd partition-broadcast -> fast
    Newton reciprocal (64 partitions) -> multiply.
"""

import sys
import types

import numpy as np


def _ensure_axon_hooks_stub():
    # bass_utils imports antenv.axon_hooks when tracing is requested (e.g. via
    # a BASS_TRACE env); the module is absent in this image. Provide a stub
    # that reports "no hook" unless a harness already installed a real one.
    if "antenv.axon_hooks" in sys.modules:
        return
    mod = types.ModuleType("antenv.axon_hooks")
    _hook = [None]
    mod.set_axon_ntff_profile_hook = lambda h: _hook.__setitem__(0, h)
    mod.get_axon_ntff_profile_hook = lambda: _hook[0]
    sys.modules["antenv.axon_hooks"] = mod
    try:
        import antenv

        antenv.axon_hooks = mod
    except ImportError:
        pass


_ensure_axon_hooks_stub()

import concourse.mybir as mybir  # noqa: E402
import concourse.tile as tile  # noqa: E402
from concourse import bacc  # noqa: E402
from concourse.bass import ts  # noqa: E402
from concourse.bass_utils import run_bass_kernel_spmd  # noqa: E402

P = 128
B, T, D = 4, 2048, 1024
H, HD = 16, 64
HG = 8          # heads per group (per core)
DG = HG * HD    # 512 channels per group
KO = D // P    # 8 contraction chunks for the projections
TQ = 512        # token chunk (attention q tile and QKV free dim)
NQT = T // TQ   # 4
F32 = mybir.dt.float32
F32R = mybir.dt.float32r

_PROGRAM = None


def _merge(attn_items, filler_items, start_frac=0.2):
    """Spread filler emission evenly between attention items, starting a bit
    into the stream so fillers' own input loads (still finishing from the
    previous stream) don't block the in-order PE queue."""
    out = []
    na, nf = len(attn_items), len(filler_items)
    lead = int(na * start_frac)
    span = max(na - lead, 1)
    fi = 0
    for i, a in enumerate(attn_items):
        out.append(a)
        j = i - lead + 1
        while fi < nf and j > 0 and j * nf >= (fi + 1) * span:
            out.append(filler_items[fi])
            fi += 1
    out.extend(filler_items[fi:])
    return out


def _build_program():
    nc = bacc.Bacc(None, target_bir_lowering=False, debug=False)

    xT = nc.dram_tensor("xT", [D, T], F32R, kind="ExternalInput")
    wqT = nc.dram_tensor("wqT", [D, DG], F32R, kind="ExternalInput")
    wkT = nc.dram_tensor("wkT", [D, DG], F32R, kind="ExternalInput")
    wvT = nc.dram_tensor("wvT", [D, DG], F32R, kind="ExternalInput")
    wpT = nc.dram_tensor("wpT", [DG, D], F32R, kind="ExternalInput")
    dmask = nc.dram_tensor("dmask", [P, 128], F32R, kind="ExternalInput")
    amask = nc.dram_tensor("amask", [P, T // P], F32, kind="ExternalInput")
    out = nc.dram_tensor("out", [T, D], F32, kind="ExternalOutput")

    xT3 = xT.ap().rearrange("(ko p) t -> p ko t", p=P)
    wq3 = wqT.ap().rearrange("(ko p) c -> p ko c", p=P)
    wk3 = wkT.ap().rearrange("(ko p) c -> p ko c", p=P)
    wv3 = wvT.ap().rearrange("(ko p) c -> p ko c", p=P)
    wp3 = wpT.ap().rearrange("(co p) d -> p co d", p=P)

    with tile.TileContext(nc) as tc:
        with tc.tile_pool(name="const", bufs=1) as cpool, \
             tc.tile_pool(name="w", bufs=1) as wpool, \
             tc.tile_pool(name="kgp", bufs=4) as kgp, \
             tc.tile_pool(name="vap", bufs=4) as vap, \
             tc.tile_pool(name="qgp", bufs=2) as qpool, \
             tc.tile_pool(name="xp", bufs=1) as xpool, \
             tc.tile_pool(name="attn", bufs=2) as apool, \
             tc.tile_pool(name="expp", bufs=4) as epool, \
             tc.tile_pool(name="divp", bufs=1) as dpool, \
             tc.tile_pool(name="outp", bufs=1) as opool, \
             tc.tile_pool(name="flow", bufs=2, space="PSUM") as flow, \
             tc.tile_pool(name="scp", bufs=2, space="PSUM") as scp, \
             tc.tile_pool(name="avp", bufs=2, space="PSUM") as avp:

            dmask_sb = cpool.tile([P, 128], F32R, tag="dmask")
            nc.sync.dma_start(dmask_sb[:], dmask.ap())
            amask_sb = cpool.tile([P, T // P], F32, tag="amask")
            nc.sync.dma_start(amask_sb[:], amask.ap())

            wq_sb = wpool.tile([P, KO, DG], F32R, tag="wq")
            wk_sb = wpool.tile([P, KO, DG], F32R, tag="wk")
            wv_sb = wpool.tile([P, KO, DG], F32R, tag="wv")
            wp_sb = wpool.tile([P, DG // P, D], F32R, tag="wp")
            # Both HWDGE queues stream concurrently; wq + x(0) land first so
            # the first q-projection matmuls start as early as possible.
            for kk in range(KO):
                eng = nc.sync if kk % 2 == 0 else nc.scalar
                eng.dma_start(wq_sb[:, kk], wq3[:, kk])
            x0_sb = xpool.tile([P, KO, TQ], F32R, tag="x", name="x0")
            for kk in range(KO):
                eng = nc.sync if kk % 2 == 1 else nc.scalar
                eng.dma_start(x0_sb[:, kk], xT3[:, kk, ts(0, TQ)])
            for kk in range(KO):
                eng = nc.sync if kk % 2 == 0 else nc.scalar
                eng.dma_start(wk_sb[:, kk], wk3[:, kk])
            for kk in range(KO):
                eng = nc.sync if kk % 2 == 1 else nc.scalar
                eng.dma_start(wv_sb[:, kk], wv3[:, kk])
            for co in range(DG // P):
                eng = nc.sync if co % 2 == 0 else nc.scalar
                eng.dma_start(wp_sb[:, co], wp3[:, co])

            kg = [None] * NQT     # per-chunk kT tiles [P, hp, TQ]
            va = [None] * NQT     # per-chunk v_aug tiles [P, h, kt2, 65]
            qg = [None] * NQT
            attn_qt = [None] * NQT
            mask_rr = [0]         # round-robin DVE/GpSimd for mask multiplies

            def qkv_items(tc4, x_pre=None):
                """QKV projection for 512-token chunk tc4, as emission items."""
                items = []
                if x_pre is not None:
                    x_sb = x_pre
                else:
                    x_sb = xpool.tile([P, KO, TQ], F32R, tag="x", name=f"x{tc4}")
                qg[tc4] = qpool.tile([P, NQT, TQ], F32R, tag="qg", name=f"qg{tc4}")
                kg[tc4] = kgp.tile([P, NQT, TQ], F32R, tag="kg", name=f"kg{tc4}")
                va[tc4] = vap.tile([P, HG, NQT, HD + 1], F32R, tag="va", name=f"va{tc4}")

                def load_x():
                    for kk in range(KO):
                        nc.sync.dma_start(x_sb[:, kk], xT3[:, kk, ts(tc4, TQ)])
                if x_pre is None:
                    items.append(load_x)

                def qk_group(w_sb, dst, cc):
                    def go():
                        ps = flow.tile([P, TQ], F32, tag="flow")
                        for kk in range(KO):
                            nc.tensor.matmul(
                                ps[:], w_sb[:, kk, ts(cc, P)], x_sb[:, kk],
                                start=(kk == 0), stop=(kk == KO - 1),
                            )
                        nc.vector.tensor_copy(dst[:, cc, :], ps[:])
                    return go

                def v_group(tt2):
                    def go():
                        ps = flow.tile([P, HG, HD], F32, tag="flow")
                        for kk in range(KO):
                            nc.tensor.matmul(
                                ps.rearrange("p h d -> p (h d)"),
                                x_sb[:, kk, ts(tt2, P)],
                                wv_sb[:, kk],
                                start=(kk == 0), stop=(kk == KO - 1),
                            )
                        am = amask_sb[:, 4 * tc4 + tt2 : 4 * tc4 + tt2 + 1]
                        nc.vector.tensor_scalar_mul(
                            va[tc4][:, :, tt2, 0:HD], ps[:], am,
                        )
                        nc.vector.tensor_copy(
                            va[tc4][:, :, tt2, HD : HD + 1],
                            am[:, None, :].to_broadcast([P, HG, 1]),
                        )
                    return go

                for cc in range(NQT):
                    items.append(qk_group(wq_sb, qg[tc4], cc))
                for cc in range(NQT):
                    items.append(qk_group(wk_sb, kg[tc4], cc))
                for tt2 in range(NQT):
                    items.append(v_group(tt2))
                return items

            def attn_hp_items(qt, hp):
                """Attention for (q chunk qt, head pair hp), software-pipelined:
                scores+exp for kt are emitted one step ahead of the AV matmuls
                for kt-1, so the PE never sits directly behind exp."""
                items = []
                if attn_qt[qt] is None:
                    attn_qt[qt] = apool.tile(
                        [P, NQT, TQ], F32R, tag="attn", name=f"attn{qt}")
                nkt = 4 * (qt + 1)
                av = [
                    avp.tile([P, TQ], F32, tag="av", name=f"av{qt}_{hp}_{par}")
                    for par in range(2)
                ]
                ex = [None] * nkt

                def scores(kt, ex=ex):
                    def go():
                        sc = scp.tile([P, 2, TQ], F32, tag="sc")
                        for par in range(2):
                            rows = slice(64 * par, 64 * par + 64)
                            nc.tensor.matmul(
                                sc[:, par],
                                kg[kt // 4][rows, hp, ts(kt % 4, P)],
                                qg[qt][rows, hp, :],
                                start=True, stop=True,
                            )
                        e = epool.tile([P, 2, TQ], F32R, tag="exp")
                        o = kt - 4 * qt
                        if o < 0:
                            nc.scalar.activation(
                                e[:], sc[:],
                                mybir.ActivationFunctionType.Exp, scale=0.125,
                            )
                        else:
                            # diagonal: exp only live columns, staircase-mask
                            # the 128-wide triangle, zero the dead columns
                            c0 = 128 * o
                            nc.scalar.activation(
                                e[:, :, c0:TQ], sc[:, :, c0:TQ],
                                mybir.ActivationFunctionType.Exp, scale=0.125,
                            )
                            tri = dmask_sb[:, 0:128]
                            nc.vector.tensor_tensor(
                                e[:, :, c0 : c0 + 128], e[:, :, c0 : c0 + 128],
                                tri[:, None, :].to_broadcast([P, 2, 128]),
                                mybir.AluOpType.mult,
                            )
                            if o > 0:
                                nc.vector.memset(e[:, :, 0:c0].bitcast(mybir.dt.uint32), 0)
                        ex[kt] = e
                    return go

                def avmm(kt, av=av, ex=ex):
                    def go():
                        for par in range(2):
                            nc.tensor.matmul(
                                av[par][: HD + 1, :],
                                va[kt // 4][:, 2 * hp + par, kt % 4, :],
                                ex[kt][:, par],
                                start=(kt == 0), stop=(kt == nkt - 1),
                            )
                    return go

                def chain(fns):
                    def go():
                        for f in fns:
                            f()
                    return go

                items.append(scores(0))
                for kt in range(1, nkt):
                    items.append(chain([scores(kt), avmm(kt - 1)]))
                items.append(avmm(nkt - 1))

                def division():
                    def go():
                        for par in range(2):
                            den = dpool.tile([1, TQ], F32, tag="den")
                            nc.vector.tensor_copy(den[:], av[par][HD : HD + 1, :])
                            rb = dpool.tile([HD, TQ], F32, tag="rb")
                            nc.gpsimd.partition_broadcast(rb[:], den[:], channels=HD)
                            rec = dpool.tile([HD, TQ], F32, tag="rec")
                            nc.vector.reciprocal_approx_fast(rec[:], rb[:])
                            nc.vector.tensor_tensor(
                                attn_qt[qt][slice(64 * par, 64 * par + 64), hp, :],
                                av[par][0:HD, :], rec[:],
                                mybir.AluOpType.mult,
                            )
                    return go

                items.append(division())
                return items

            def outproj_items(qt):
                """Output projection for q chunk qt."""
                items = []

                def tt_group(tt2):
                    def go():
                        o_sb = opool.tile([P, D], F32, tag="osb")
                        for nb in range(D // TQ):
                            ps = flow.tile([P, TQ], F32, tag="flow")
                            for cc in range(DG // P):
                                nc.tensor.matmul(
                                    ps[:],
                                    attn_qt[qt][:, cc, ts(tt2, P)],
                                    wp_sb[:, cc, ts(nb, TQ)],
                                    start=(cc == 0), stop=(cc == DG // P - 1),
                                )
                            nc.vector.tensor_copy(o_sb[:, ts(nb, TQ)], ps[:])
                        nc.sync.dma_start(
                            out.ap()[ts(qt * NQT + tt2, P), :], o_sb[:]
                        )
                    return go

                for tt2 in range(NQT):
                    items.append(tt_group(tt2))
                return items

            # Emission schedule (engine queues execute in emission order, so
            # PE-filler work is placed where attention would stall on exp):
            #   qkv(0) | attn(0) x qkv(1) | attn(1) x [qkv(2), op(0)]
            #   | attn(2) x qkv(3) | attn(3,hp01) x op(1)
            #   | attn(3,hp23) x op(2) | op(3)
            def attn_qt_items(qt, hps):
                items = []
                for hp in hps:
                    items += attn_hp_items(qt, hp)
                return items

            for it in qkv_items(0, x_pre=x0_sb):
                it()
            for it in _merge(attn_qt_items(0, range(4)), qkv_items(1)):
                it()
            for it in _merge(attn_qt_items(1, range(4)),
                             qkv_items(2) + outproj_items(0)):
                it()
            for it in _merge(attn_qt_items(2, range(4)), qkv_items(3)):
                it()
            for it in _merge(attn_qt_items(3, [0, 1]), outproj_items(1)):
                it()
            for it in _merge(attn_qt_items(3, [2, 3]), outproj_items(2)):
                it()
            for it in outproj_items(NQT - 1):
                it()

    nc.compile()
    return nc


def _get_program():
    global _PROGRAM
    if _PROGRAM is None:
        _PROGRAM = _build_program()
    return _PROGRAM


def _staircase_mask() -> np.ndarray:
    # dmask[i, j] = 1.0 iff j >= i + 384; the slice [:, 384-128*o : 512]
    # give the four diagonal-block staircase masks.
    i = np.arange(P)[:, None]
    j = np.arange(128)[None, :]
    return (j >= i).astype(np.float32)


def make_in_maps(x, attention_mask, w_qkv, w_proj):
    x = np.asarray(x, dtype=np.float32)
    attention_mask = np.asarray(attention_mask)
    w_qkv = np.asarray(w_qkv, dtype=np.float32)
    w_proj = np.asarray(w_proj, dtype=np.float32)
    dm = _staircase_mask()
    in_maps = []
    for c in range(8):
        g, b = c // 4, c % 4
        rows = slice(DG * g, DG * g + DG)
        in_maps.append({
            "xT": np.ascontiguousarray(x[b].T),
            "wqT": np.ascontiguousarray(w_qkv[0 * D :][rows].T),
            "wkT": np.ascontiguousarray(w_qkv[1 * D :][rows].T),
            "wvT": np.ascontiguousarray(w_qkv[2 * D :][rows].T),
            "wpT": np.ascontiguousarray(w_proj[:, rows].T),
            "dmask": dm,
            "amask": np.ascontiguousarray(
                attention_mask[b].astype(np.float32).reshape(T // P, P).T
            ),
        })
    return in_maps


def run_spmd(in_maps, **kwargs):
    nc = _get_program()
    return run_bass_kernel_spmd(nc, in_maps, list(range(8)), **kwargs)


def kernel(x, attention_mask, w_qkv, w_proj, n_heads):
    assert int(n_heads) == H
    in_maps = make_in_maps(x, attention_mask, w_qkv, w_proj)
    res = run_spmd(in_maps)
    parts = [res.results[c]["out"] for c in range(8)]
    return np.stack([parts[b] + parts[b + 4] for b in range(B)]).astype(np.float32)
